# revision 27
# baseline (speedup 1.0000x reference)
"""Trainium2 Bass kernel for nn_CausalSelfAttention_17248588661518.

Causal self-attention (B=2, T=2048, C=1024, H=16) with a FIRE relative
position bias produced by a tiny MLP: bias[h,t,s] = relu(nd*w1+b1) @ w2 + b2
where nd = log(|c*(t-s)|+1) / (log(|c*max(t,thr)|+1)+eps).

Sharding: tensor-parallel over heads — each of the 8 cores owns 2 heads.
Each core computes the QKV projection for its head columns, its heads'
attention, and a column-parallel partial of the output projection; the host
sums the 8 partial projections (the tensor-parallel all-reduce) and adds
bproj.

Device math (valid because b1 == 0 and nd >= 0, both guaranteed by the
input spec fills):
    relu(nd * w1[w]) == nd * max(w1[w], 0)
so  bias_h = A_h * nd + b2_h with A_h = sum_w max(w1[w],0) * w2[w,h].
The host precomputes ND[s,t] = log(|c|(t-s)+1) * invPn[t] (zeroed for t<s)
once; on device the bias lands in PSUM via a second accumulating matmul
with a scaled identity (A_h * I) as the stationary operand.

Layouts (per core):
    qT, kT : (128 = 2 heads x 64, B*T) fp32, d on partitions (from the QKV
             matmul with the weight slice as stationary operand)
    v      : (128 s x 130) bf16 tiles per (b, s-tile): [v_h0 | 1 | v_h1 | 1]
             (ones column yields the softmax denominator inside the AV matmul)
    att    : (128 s x 512 t) PSUM tiles; softmax has no max-subtraction
             (logits are bounded, ~[-3.1, 2.9] for the spec inputs)
    yT     : (65 x 512) PSUM accumulators per t-chunk; row 64 = sum of exp.
Causal masking is free: the AV matmul simply restricts its moving-operand
columns to t >= s; the masked region of exp(att) is never read.
"""

import os
from contextlib import ExitStack

import numpy as np
import ml_dtypes

import concourse.bass as bass
import concourse.mybir as mybir
from concourse import bacc
from concourse.tile import TileContext
from concourse.bass_utils import run_bass_kernel_spmd

B, T, C = 2, 2048, 1024
H, HD = 16, 64
NCORES = 8
BT = B * T
NST = T // 128            # s-tiles per batch
NJC = T // 512            # 512-wide t-chunks per batch
F32 = mybir.dt.float32
F32R = mybir.dt.float32r
BF16 = mybir.dt.bfloat16
F16 = mybir.dt.float16
EXP = mybir.ActivationFunctionType.Exp

# causal (s-tile, t-chunk) pairs within one batch and their packed index
TILES = [(i, j) for i in range(NST) for j in range(i // 4, NJC)]
TIDX = {t: n for n, t in enumerate(TILES)}
NTILES = len(TILES)  # 40

_prog_cache = {}
DEBUG_DUMPS = False


def build_program():
    nc = bacc.Bacc(
        "TRN2",
        target_bir_lowering=False,
        debug=False,
        enable_asserts=False,
        num_devices=NCORES,
    )
    xtb = nc.dram_tensor("xtb", [C, BT], BF16, kind="ExternalInput")
    wqk = nc.dram_tensor("wqk", [C, 384], BF16, kind="ExternalInput")
    ndm = nc.dram_tensor("ndm", [128, NTILES * 512], BF16, kind="ExternalInput")
    aim = nc.dram_tensor("aim", [128, 256], BF16, kind="ExternalInput")
    b2b = nc.dram_tensor("b2b", [128, 2], F32, kind="ExternalInput")
    wp = nc.dram_tensor("wp", [128, C], BF16, kind="ExternalInput")
    trim = nc.dram_tensor("trim", [128, 128], BF16, kind="ExternalInput")
    out = nc.dram_tensor("out", [BT, C], F16, kind="ExternalOutput")

    xtb_r = xtb[:].rearrange("(o p) t -> p o t", p=128)
    wqk_r = wqk[:].rearrange("(o p) j -> p o j", p=128)

    with TileContext(nc) as tc, ExitStack() as ctx:
        cpool = ctx.enter_context(tc.tile_pool(name="consts", bufs=1))
        spool = ctx.enter_context(tc.tile_pool(name="state", bufs=1))
        xpool = ctx.enter_context(tc.tile_pool(name="xstream", bufs=2))
        ppool = ctx.enter_context(tc.tile_pool(name="pbuf", bufs=6))
        ytpool = ctx.enter_context(tc.tile_pool(name="ytbuf", bufs=2))
        opool = ctx.enter_context(tc.tile_pool(name="obuf", bufs=3))
        mpool = ctx.enter_context(tc.tile_pool(name="misc", bufs=2))
        ps = ctx.enter_context(tc.tile_pool(name="ps", bufs=4, space="PSUM"))
        psyt = ctx.enter_context(tc.tile_pool(name="psyt", bufs=4, space="PSUM"))

        wqk_sb = cpool.tile([128, 8, 384], BF16)
        nc.sync.dma_start(wqk_sb[:], wqk_r)
        ident = cpool.tile([128, 128], BF16)

        q_sb = spool.tile([128, BT], BF16)
        k_sb = spool.tile([128, BT], BF16)
        v_sb = spool.tile([128, 2 * NST, 256], BF16)

        # ---- Phase 1: QKV projections (q, k, vT) + v transpose ------------
        from concourse.masks import make_identity

        make_identity(nc, ident[:])
        vt_sb = spool.tile([128, BT], BF16)
        for tch in range(BT // 512):
            tsl = slice(tch * 512, (tch + 1) * 512)
            xtb_t = xpool.tile([128, 8, 512], BF16, tag="xtb", name="xtb_t")
            nc.sync.dma_start(xtb_t[:], xtb_r[:, :, tsl])
            for j in range(3):  # 0 -> q, 1 -> k, 2 -> v columns
                qk_ps = ps.tile([128, 512], F32, tag="att", name="qk_ps")
                for m in range(8):
                    nc.tensor.matmul(
                        qk_ps[:],
                        wqk_sb[:, m, j * 128 : (j + 1) * 128],
                        xtb_t[:, m, :],
                        start=(m == 0),
                        stop=(m == 7),
                    )
                dst = (q_sb, k_sb, vt_sb)[j]
                nc.vector.tensor_copy(dst[:, tsl], qk_ps[:])
        # transpose vT (d x s) -> v (s x d) per 128-token block
        for ig in range(2 * NST):
            tp = ps.tile([128, 128], BF16, tag="att", name="tp")
            nc.tensor.transpose(
                tp[:], vt_sb[:, ig * 128 : (ig + 1) * 128], ident[:]
            )
            nc.vector.tensor_copy(v_sb[:, ig, 0:64], tp[:, 0:64])
            nc.vector.tensor_copy(v_sb[:, ig, 128:192], tp[:, 64:128])
        nc.vector.memset(v_sb[:, :, 64:65], 1.0)
        nc.vector.memset(v_sb[:, :, 192:193], 1.0)
        nc.vector.memset(v_sb[:, :, 65:128], 0.0)
        nc.vector.memset(v_sb[:, :, 193:256], 0.0)

        ai_sb = cpool.tile([128, 256], BF16)
        nc.scalar.dma_start(ai_sb[:], aim[:])
        b2_sb = cpool.tile([128, 2], F32)
        nc.scalar.dma_start(b2_sb[:], b2b[:])
        trim_sb = cpool.tile([128, 128], BF16)
        nc.scalar.dma_start(trim_sb[:], trim[:])
        wp_sb = cpool.tile([128, C], BF16)
        nc.scalar.dma_start(wp_sb[:], wp[:])
        nd_sb = cpool.tile([128, NTILES, 512], BF16)
        nc.scalar.dma_start(nd_sb[:].rearrange("p a b -> p (a b)"), ndm[:])

        # ---- Phase 2: attention, both heads interleaved, j-major ----------
        for b in range(B):
            yt_sb = ytpool.tile([128, T], BF16, tag="yt", name="yt_sb")

            def _evac_yt(yt_ps_j, j, hl, b=b):
                koff = hl * 64
                sums_sb = mpool.tile([1, 512], F32, tag="sums", name="sums_sb")
                nc.vector.tensor_copy(sums_sb[:], yt_ps_j[64:65, :])
                rec = mpool.tile([1, 512], F32, tag="rec", name="rec")
                scr = mpool.tile([1, 512], F32, tag="scr", name="scr")
                nc.vector.reciprocal_approx_accurate(
                    out=rec[:], in_=sums_sb[:], scratch=scr[:]
                )
                bc = mpool.tile([64, 512], F32, tag="bc", name="bc")
                nc.gpsimd.partition_broadcast(bc[:], rec[:])
                nc.vector.tensor_mul(
                    yt_sb[koff : koff + 64, j * 512 : (j + 1) * 512],
                    yt_ps_j[0:64, :],
                    bc[:],
                )

            for j in range(NJC):
                yt_ps = [
                    psyt.tile([128, 512], F32, tag="ytps", name=f"ytps{hl}")
                    for hl in range(2)
                ]
                pending = None

                def _emit_av(pi, p_pair, j=j):
                    off = max(0, pi * 128 - j * 512)
                    for hl in range(2):
                        nc.tensor.matmul(
                            yt_ps[hl][:, off:512],
                            v_sb[:, b * NST + pi, hl * 128 : (hl + 1) * 128],
                            p_pair[hl][:, off:512],
                            start=(pi == 0),
                            stop=(pi == 4 * j + 3),
                        )

                for i in range(4 * j + 4):
                    off = max(0, i * 128 - j * 512)
                    atts = []
                    for hl in range(2):
                        att = ps.tile([128, 512], F32, tag="att", name="att")
                        nc.tensor.matmul(
                            att[:, :],
                            k_sb[
                                hl * 64 : hl * 64 + 64,
                                b * T + i * 128 : b * T + (i + 1) * 128,
                            ],
                            q_sb[
                                hl * 64 : hl * 64 + 64,
                                b * T + j * 512 : b * T + (j + 1) * 512,
                            ],
                            start=True,
                            stop=False,
                        )
                        nc.tensor.matmul(
                            att[:, :],
                            ai_sb[:, hl * 128 : (hl + 1) * 128],
                            nd_sb[:, TIDX[(i, j)], :],
                            start=False,
                            stop=True,
                        )
                        atts.append(att)
                    p_pair = []
                    for hl in range(2):
                        p_t = ppool.tile([128, 512], BF16, tag="p", name="p_t")
                        nc.scalar.activation(
                            p_t[:, off:512],
                            atts[hl][:, off:512],
                            EXP,
                            bias=b2_sb[:, hl : hl + 1],
                            scale=1.0,
                        )
                        p_pair.append(p_t)
                    if i >= 4 * j:  # diagonal block: triangular mask
                        for hl in range(2):
                            nc.vector.tensor_mul(
                                p_pair[hl][:, off : off + 128],
                                p_pair[hl][:, off : off + 128],
                                trim_sb[:],
                            )
                    if pending is not None:
                        _emit_av(*pending)
                    pending = (i, p_pair)
                _emit_av(*pending)
                for hl in range(2):
                    _evac_yt(yt_ps[hl], j, hl)
            # ---- Phase 3: partial output projection for batch b ----------
            for tcq in range(NST):
                o_sb = opool.tile([128, C], F16, tag="o", name="o_sb")
                for nh in range(2):
                    pp = ps.tile([128, 512], F32, tag="att", name="pp")
                    nc.tensor.matmul(
                        pp[:],
                        yt_sb[:, tcq * 128 : (tcq + 1) * 128],
                        wp_sb[:, nh * 512 : (nh + 1) * 512],
                        start=True,
                        stop=True,
                    )
                    nc.vector.tensor_copy(o_sb[:, nh * 512 : (nh + 1) * 512], pp[:])
                nc.sync.dma_start(
                    out[b * T + tcq * 128 : b * T + (tcq + 1) * 128, :], o_sb[:]
                )
        if DEBUG_DUMPS:
            dq = nc.dram_tensor("dbg_q", [128, BT], BF16, kind="ExternalOutput")
            dk = nc.dram_tensor("dbg_k", [128, BT], BF16, kind="ExternalOutput")
            dv = nc.dram_tensor("dbg_v", [128, 2 * NST * 256], BF16, kind="ExternalOutput")
            dyt = nc.dram_tensor("dbg_yt", [128, T], BF16, kind="ExternalOutput")
            nc.sync.dma_start(dq[:], q_sb[:])
            nc.sync.dma_start(dk[:], k_sb[:])
            nc.sync.dma_start(dv[:], v_sb[:].rearrange("p a b -> p (a b)"))
            nc.sync.dma_start(dyt[:], yt_sb[:])
    nc.finalize()
    return nc


def get_program():
    if "nc" not in _prog_cache:
        _prog_cache["nc"] = build_program()
    return _prog_cache["nc"]


def _host_prep(x, Wqkv, Wproj, w1, w2, b2, c_param, L_multiplier):
    """Builds shared and per-core device inputs."""
    f = np.float64
    c = abs(float(c_param))
    thr = abs(float(L_multiplier) * 512.0)
    pos = np.arange(T, dtype=f)
    R = np.log(c * pos + 1.0)
    invPn = 1.0 / (np.log(c * np.maximum(pos, thr) + 1.0) + 1e-6)
    idx = np.arange(T)[None, :] - np.arange(T)[:, None]  # t - s, (s, t)
    nd_full = np.where(idx >= 0, R[np.clip(idx, 0, T - 1)] * invPn[None, :], 0.0)
    ndm = np.empty((128, NTILES * 512), np.float32)
    for (i, j), n in TIDX.items():
        ndm[:, n * 512 : (n + 1) * 512] = nd_full[
            i * 128 : (i + 1) * 128, j * 512 : (j + 1) * 512
        ]
    ndm = ndm.astype(ml_dtypes.bfloat16)

    A = (np.maximum(w1[0].astype(f), 0.0) @ w2.astype(f)).astype(np.float32)  # (H,)
    scale = 1.0 / np.sqrt(HD)

    xtb = np.ascontiguousarray(x.reshape(BT, C).T.astype(ml_dtypes.bfloat16))

    eye = np.eye(128, dtype=np.float32)
    trim = np.triu(np.ones((128, 128), np.float32)).astype(ml_dtypes.bfloat16)
    in_maps = []
    for core in range(NCORES):
        h0 = 2 * core
        qcols = Wqkv[:, h0 * HD : (h0 + 2) * HD].astype(np.float32) * scale
        kcols = Wqkv[:, C + h0 * HD : C + (h0 + 2) * HD].astype(np.float32)
        vcols = Wqkv[:, 2 * C + h0 * HD : 2 * C + (h0 + 2) * HD].astype(np.float32)
        wqk_all = np.concatenate([qcols, kcols, vcols], axis=1)
        ai = np.concatenate([A[h0] * eye, A[h0 + 1] * eye], axis=1)
        b2c = np.broadcast_to(
            np.asarray([b2[h0], b2[h0 + 1]], np.float32)[None, :], (128, 2)
        )
        in_maps.append(
            {
                "xtb": xtb,
                "wqk": np.ascontiguousarray(wqk_all.astype(ml_dtypes.bfloat16)),
                "ndm": ndm,
                "aim": np.ascontiguousarray(ai.astype(ml_dtypes.bfloat16)),
                "b2b": np.ascontiguousarray(b2c),
                "wp": np.ascontiguousarray(
                    Wproj[core * 128 : (core + 1) * 128, :].astype(ml_dtypes.bfloat16)
                ),
                "trim": trim,
            }
        )
    return in_maps


def _gather(results, bproj):
    acc = np.zeros((BT, C), np.float32)
    for r in results:
        acc += r["out"].astype(np.float32)
    acc += bproj.astype(np.float32)[None, :]
    return acc.reshape(B, T, C)


def _numpy_fallback(x, Wqkv, bqkv, Wproj, bproj, w1, b1, w2, b2, c_param, L_multiplier):
    """Exact (slow) host fallback for inputs violating the spec fills."""
    f = np.float64
    c = float(c_param)
    thr = abs(float(L_multiplier) * 512.0)
    pos = np.arange(T, dtype=f)
    rel = np.log(np.abs(c * (pos[:, None] - pos[None, :])) + 1.0)  # (t, s)
    pn = np.log(np.abs(c * np.maximum(pos, thr)) + 1.0) + 1e-6
    nd = rel / pn[:, None]
    qkv = x.reshape(BT, C).astype(f) @ Wqkv.astype(f) + bqkv.astype(f)
    qkv = qkv.reshape(B, T, 3 * C)
    q = qkv[..., :C].reshape(B, T, H, HD)
    k = qkv[..., C : 2 * C].reshape(B, T, H, HD)
    v = qkv[..., 2 * C :].reshape(B, T, H, HD)
    causal = (pos[:, None] - pos[None, :]) >= 0  # (t, s)
    outp = np.zeros((B, T, C), f)
    hfe = np.maximum(nd[..., None] * w1[0].astype(f) + b1.astype(f), 0.0)  # (t,s,32)
    for h in range(H):
        bias = hfe @ w2[:, h].astype(f) + float(b2[h])  # (t, s)
        logits_bias = np.where(causal, bias, -np.inf)
        for b in range(B):
            att = (q[b, :, h] @ k[b, :, h].T) / np.sqrt(HD) + logits_bias
            att -= att.max(axis=1, keepdims=True)
            P = np.exp(att)
            P /= P.sum(axis=1, keepdims=True)
            outp[b] += (P @ v[b, :, h]) @ Wproj[h * HD : (h + 1) * HD].astype(f)
    outp += bproj.astype(f)
    return outp.astype(np.float32)


def run(inputs, trace=False, trace_cores=None):
    """Builds per-core inputs, runs the SPMD kernel, gathers the output.

    Returns (output, BassKernelResults)."""
    nc = get_program()
    in_maps = _host_prep(
        inputs["x"], inputs["Wqkv"], inputs["Wproj"], inputs["w1"], inputs["w2"],
        inputs["b2"], inputs["c_param"], inputs["L_multiplier"],
    )
    kwargs = {}
    if trace:
        kwargs["trace"] = True
        if trace_cores is not None:
            kwargs["trace_cores"] = trace_cores
    res = run_bass_kernel_spmd(nc, in_maps, core_ids=list(range(NCORES)), **kwargs)
    outp = _gather(res.results, np.asarray(inputs["bproj"]))
    return outp, res


def kernel(x, Wqkv, bqkv, Wproj, bproj, w1, b1, w2, b2, c_param, L_multiplier):
    inputs = dict(
        x=np.asarray(x), Wqkv=np.asarray(Wqkv), bqkv=np.asarray(bqkv),
        Wproj=np.asarray(Wproj), bproj=np.asarray(bproj), w1=np.asarray(w1),
        b1=np.asarray(b1), w2=np.asarray(w2), b2=np.asarray(b2),
        c_param=np.asarray(c_param), L_multiplier=np.asarray(L_multiplier),
    )
    if np.any(inputs["b1"]) or np.any(inputs["bqkv"]):
        # outside the spec'd zero-fill regime the device fast path is invalid
        return _numpy_fallback(**inputs)
    outp, _ = run(inputs)
    return outp
